# revision 29
# baseline (speedup 1.0000x reference)
"""Trainium2 Bass kernel for causal multi-head attention with RoPE (v2, fp8).

Problem: B=4, T=2048, C=1024, 16 heads, head_dim=64, fp32.
Sharding over 8 cores: core c handles batch c//2 and heads [8*(c%2), 8*(c%2)+8).
Each core computes a [T, C] partial of the output projection; the host sums
the two partials per batch, scales by 1/128 and adds b_proj.

Precision plan (validated by numpy emulation):
  - QKV GEMMs: fp8 e4m3 DoubleRow (256-deep contraction per instruction),
    weights host-scaled x16 into the fp8 normal range.
  - S = q^T k: plain fp8 (contraction is only 64, DoubleRow gains nothing).
  - P = exp((S - 2*2048)/2048): ACT engine straight from PSUM into fp8.
  - AV: fp8 DoubleRow over key-tile pairs; denominator via a constant-2.0
    column appended to V (row 64 of the AV output).
  - proj: bf16 (fp8 here would put ~4% RMS directly on the output).
  The net 128x output scale (16 * 8) is divided out on the host.

Head layout per core: local head h in [0,8) -> tile gg=h//2, partition base
64*(h%2); head dims stored interleaved (RoPE rotation pairs adjacent).

Self-contained: hardcodes shapes; needs numpy + ml_dtypes + concourse.
"""

import numpy as np
import ml_dtypes
from contextlib import ExitStack

import concourse.bass as bass
import concourse.tile as tile
from concourse import bacc, mybir
from concourse.bass_utils import run_bass_kernel_spmd

F32 = mybir.dt.float32
BF16 = mybir.dt.bfloat16
FP8 = mybir.dt.float8e4
AF = mybir.ActivationFunctionType
DR = mybir.MatmulPerfMode.DoubleRow

B, T, C = 4, 2048, 1024
N_HEAD = 16
HD = 64
HG = 8  # heads per core
SCALE_EFF = 1.0 / (np.sqrt(HD) * 256.0)  # wq,wk each scaled x16
EXP_BIAS = -2.0

_NC_CACHE = {}
LAST_RESULTS = None


def _pair_swap_mask():
    m = []
    for i in range(16):
        m += [2 * i + 1, 2 * i]
    return m


def mk(ap, off, dims):
    """AP with the same partition dim but custom free dims, offset in elems."""
    return bass.AP(tensor=ap.tensor, offset=ap.offset + off, ap=[list(ap.ap[0])] + dims)


def build_nc(t=T):
    key = t
    if key in _NC_CACHE:
        return _NC_CACHE[key]

    n_tt = t // 128  # 16 key tiles
    n_tb = t // 512  # 4 query blocks
    n_pair = n_tt // 2  # 8 key-tile pairs

    nc = bacc.Bacc("TRN2", target_bir_lowering=False, debug=False, num_devices=8)

    xt_d = nc.dram_tensor("xt", [128, 8 * t], FP8, kind="ExternalInput").ap()
    xb_d = nc.dram_tensor("xb", [128, 4096], BF16, kind="ExternalInput").ap()
    wq_d = nc.dram_tensor("wq", [128, 4096], FP8, kind="ExternalInput").ap()
    wk_d = nc.dram_tensor("wk", [128, 4096], FP8, kind="ExternalInput").ap()
    wv_d = nc.dram_tensor("wv", [128, 4096], FP8, kind="ExternalInput").ap()
    wp_d = nc.dram_tensor("wp", [128, 4096], BF16, kind="ExternalInput").ap()
    wqb_d = nc.dram_tensor("wqb", [128, 4096], BF16, kind="ExternalInput").ap()
    wkb_d = nc.dram_tensor("wkb", [128, 4096], BF16, kind="ExternalInput").ap()
    wvb_d = nc.dram_tensor("wvb", [128, 4096], BF16, kind="ExternalInput").ap()
    bq_d = nc.dram_tensor("bq", [128, 4], F32, kind="ExternalInput").ap()
    bk_d = nc.dram_tensor("bk", [128, 4], F32, kind="ExternalInput").ap()
    bv_d = nc.dram_tensor("bv", [512], F32, kind="ExternalInput").ap()
    cos_d = nc.dram_tensor("cosT", [128, t], BF16, kind="ExternalInput").ap()
    sin_d = nc.dram_tensor("sinS", [128, t], BF16, kind="ExternalInput").ap()
    out_d = nc.dram_tensor("out", [t, C], F32, kind="ExternalOutput").ap()

    shuf = _pair_swap_mask()

    with tile.TileContext(nc) as tc, ExitStack() as ctx:
        # ---------------- persistent SBUF ----------------
        persist = ctx.enter_context(tc.tile_pool(name="persist", bufs=1))
        xt = persist.tile([128, 8 * t], FP8, tag="xt", name="xt")
        xb = persist.tile([128, 4096], BF16, tag="xb", name="xb")  # x^T keys<512
        qt8 = [persist.tile([128, t], BF16, tag=f"qt{g}", name=f"qt{g}") for g in range(4)]
        kt8 = [persist.tile([128, t], BF16, tag=f"kt{g}", name=f"kt{g}") for g in range(4)]
        vp = [persist.tile([128, 1280], FP8, tag=f"v{i}", name=f"v{i}") for i in range(n_pair)]
        vb = [persist.tile([128, 520], BF16, tag=f"vb{i}", name=f"vb{i}") for i in range(4)]
        yt = persist.tile([128, 4 * t], BF16, tag="yt", name="yt")
        wq_sb = persist.tile([128, 4096], FP8, tag="wq", name="wq")
        wk_sb = persist.tile([128, 4096], FP8, tag="wk", name="wk")
        wv_sb = persist.tile([128, 4096], FP8, tag="wv", name="wv")
        wp_sb = persist.tile([128, 4096], BF16, tag="wp", name="wp")
        wqb_sb = persist.tile([128, 4096], BF16, tag="wqb", name="wqb")
        wkb_sb = persist.tile([128, 4096], BF16, tag="wkb", name="wkb")
        wvb_sb = persist.tile([128, 4096], BF16, tag="wvb", name="wvb")
        cos_sb = persist.tile([128, t], BF16, tag="cos", name="cos")
        sin_sb = persist.tile([128, t], BF16, tag="sin", name="sin")
        bq_sb = persist.tile([128, 4], F32, tag="bq", name="bq")
        bk_sb = persist.tile([128, 4], F32, tag="bk", name="bk")
        bv_sb = persist.tile([128, 512], F32, tag="bv", name="bv")
        ones_bc = persist.tile([128, 64], BF16, tag="ones", name="ones")
        nbias = persist.tile([128, 1], F32, tag="nbias", name="nbias")

        scr = ctx.enter_context(tc.tile_pool(name="scr", bufs=3))
        rtp = ctx.enter_context(tc.tile_pool(name="rtp", bufs=2))
        pP = ctx.enter_context(tc.tile_pool(name="pP", bufs=3))
        pPb = ctx.enter_context(tc.tile_pool(name="pPb", bufs=2))
        ytp = ctx.enter_context(tc.tile_pool(name="ytp", bufs=8))
        recp = ctx.enter_context(tc.tile_pool(name="recp", bufs=8))
        osb = ctx.enter_context(tc.tile_pool(name="osb", bufs=4))

        psA = ctx.enter_context(tc.tile_pool(name="psA", bufs=2, space="PSUM"))
        psS = ctx.enter_context(tc.tile_pool(name="psS", bufs=2, space="PSUM"))
        psAV = ctx.enter_context(tc.tile_pool(name="psAV", bufs=1, space="PSUM"))

        nc.vector.memset(ones_bc[:], 1.0)
        nc.vector.memset(nbias[:], EXP_BIAS)

        # ---------------- input DMAs, spread across queues ----------------
        nc.gpsimd.dma_start(wvb_sb[:], wvb_d)
        nc.sync.dma_start(xb[:], xb_d)
        nc.gpsimd.dma_start(wv_sb[:], wv_d)
        for i in range(4):
            nc.sync.dma_start(xt[:, i * 4096 : (i + 1) * 4096], xt_d[:, i * 4096 : (i + 1) * 4096])
        nc.scalar.dma_start(wq_sb[:], wq_d)
        nc.scalar.dma_start(wk_sb[:], wk_d)
        nc.scalar.dma_start(wqb_sb[:], wqb_d)
        nc.scalar.dma_start(wkb_sb[:], wkb_d)
        nc.scalar.dma_start(cos_sb[:], cos_d)
        nc.scalar.dma_start(sin_sb[:], sin_d)
        nc.scalar.dma_start(bq_sb[:], bq_d)
        nc.scalar.dma_start(bk_sb[:], bk_d)
        nc.sync.dma_start(
            bv_sb[:], bass.AP(tensor=bv_d.tensor, offset=0, ap=[[0, 128], [1, 512]])
        )
        nc.gpsimd.dma_start(wp_sb[:], wp_d)

        # 2.0 columns for the softmax denominator (d=64 slot per (h, j))
        for p in range(n_pair):
            nc.gpsimd.memset(mk(vp[p][:], 64, [[160, 8], [80, 2]]), 2.0)
        for i in range(4):
            nc.gpsimd.memset(mk(vb[i][:], 64, [[65, 8]]), 2.0)

        # ---------------- V: per key tile, 2 channel blocks ----------------
        # keys < 512: accurate bf16 GEMM, written to both vb (bf16) and vp
        # (fp8); keys >= 512: fp8 DoubleRow.
        for kt in range(n_tt):
            for cb in range(2):
                ps = psA.tile([128, 256], F32, tag="mm", name="psv")
                if kt < 4:
                    for ci in range(8):
                        nc.tensor.matmul(
                            ps[:],
                            mk(xb[:], ci * 512 + kt * 128, [[1, 128]]),
                            mk(wvb_sb[:], ci * 512 + cb * 256, [[1, 256]]),
                            start=(ci == 0),
                            stop=(ci == 7),
                        )
                    nc.vector.tensor_add(
                        mk(vb[kt][:], (4 * cb) * 65, [[65, 4], [1, 64]]),
                        mk(ps[:], 0, [[64, 4], [1, 64]]),
                        mk(bv_sb[:], cb * 256, [[64, 4], [1, 64]]),
                    )
                else:
                    for p in range(4):
                        nc.tensor.matmul(
                            ps[:],
                            mk(xt[:], (2 * p) * t + kt * 128, [[t, 2], [1, 128]]),
                            mk(wv_sb[:], cb * 2048 + p * 512, [[256, 2], [1, 256]]),
                            start=(p == 0),
                            stop=(p == 3),
                            perf_mode=DR,
                        )
                nc.vector.tensor_add(
                    mk(vp[kt // 2][:], (4 * cb) * 160 + (kt % 2) * 80, [[160, 4], [1, 64]]),
                    mk(ps[:], 0, [[64, 4], [1, 64]]),
                    mk(bv_sb[:], cb * 256, [[64, 4], [1, 64]]),
                )

        # ---------------- Q, K with RoPE ----------------
        def qk_block(w_sb, wb_sb, b_sb, dst, gg):
            qscr = scr.tile([128, t], BF16, tag="scr", name="qscr")
            for nb in range(8):
                ps = psA.tile([128, 256], F32, tag="mm", name="psq")
                if nb < 2:
                    # queries < 512: accurate bf16 GEMM
                    for ci in range(8):
                        nc.tensor.matmul(
                            ps[:],
                            mk(wb_sb[:], gg * 1024 + ci * 128, [[1, 128]]),
                            mk(xb[:], ci * 512 + nb * 256, [[1, 256]]),
                            start=(ci == 0),
                            stop=(ci == 7),
                        )
                else:
                    for p in range(4):
                        nc.tensor.matmul(
                            ps[:],
                            mk(w_sb[:], gg * 1024 + p * 256, [[128, 2], [1, 128]]),
                            mk(xt[:], (2 * p) * t + nb * 256, [[t, 2], [1, 256]]),
                            start=(p == 0),
                            stop=(p == 3),
                            perf_mode=DR,
                        )
                nc.scalar.add(qscr[:, nb * 256 : (nb + 1) * 256], ps[:], b_sb[:, gg : gg + 1])
            tmp = rtp.tile([128, t], BF16, tag="rt", name="rtmp")
            nc.vector.stream_shuffle(tmp[:], qscr[:], shuf)
            nc.vector.tensor_mul(tmp[:], tmp[:], sin_sb[:])
            nc.vector.tensor_mul(qscr[:], qscr[:], cos_sb[:])
            nc.vector.tensor_add(dst[:], qscr[:], tmp[:])

        for gg in range(4):
            qk_block(wq_sb, wqb_sb, bq_sb, qt8[gg], gg)
            qk_block(wk_sb, wkb_sb, bk_sb, kt8[gg], gg)

        # ---------------- attention, ib-major with proj interleaved ----------
        proj_pending = []
        dma_i = [0]

        def emit_proj_chunk():
            ti, cb = proj_pending.pop(0)
            ps = psA.tile([128, 256], F32, tag="mm", name="psp")
            for ci in range(4):
                nc.tensor.matmul(
                    ps[:],
                    mk(yt[:], ci * t + ti * 128, [[1, 128]]),
                    mk(wp_sb[:], ci * 1024 + cb * 256, [[1, 256]]),
                    start=(ci == 0),
                    stop=(ci == 3),
                )
            o = osb.tile([128, 256], F32, tag="o", name="o")
            nc.vector.tensor_copy(o[:], ps[:])
            eng = nc.sync if dma_i[0] % 2 == 0 else nc.scalar
            dma_i[0] += 1
            eng.dma_start(out_d[ti * 128 : (ti + 1) * 128, cb * 256 : (cb + 1) * 256], o[:])

        for ib in range(n_tb):
            for gg in range(4):
                P_cur = None
                av = None
                n_jt = 4 * ib + 4
                for jt in range(n_jt):
                    r = jt - 4 * ib
                    c0 = max(0, 128 * r)
                    sq = psS.tile([128, 1024], F32, tag="s", name="sq")
                    for s2 in range(2):
                        nc.tensor.matmul(
                            sq[:, s2 * 512 + c0 : (s2 + 1) * 512],
                            kt8[gg][64 * s2 : 64 * s2 + 64, jt * 128 : (jt + 1) * 128],
                            qt8[gg][64 * s2 : 64 * s2 + 64, ib * 512 + c0 : (ib + 1) * 512],
                            start=True,
                            stop=True,
                            tile_position=(64 * s2, 0),
                        )
                    if ib == 0:
                        P_cur = pPb.tile([128, 1024], BF16, tag="pb", name="Pb")
                        js = 0
                    else:
                        if jt % 2 == 0:
                            P_cur = pP.tile([128, 2048], FP8, tag="p", name="P")
                        js = jt % 2
                    nc.scalar.activation(
                        mk(P_cur[:], js * 1024 + c0, [[512, 2], [1, 512 - c0]]),
                        mk(sq[:], c0, [[512, 2], [1, 512 - c0]]),
                        AF.Exp,
                        scale=SCALE_EFF,
                        bias=nbias[:],
                    )
                    if r in (1, 3):
                        # zero the consumed-but-unwritten 128 cols below c0
                        nc.gpsimd.memset(
                            mk(P_cur[:], js * 1024 + c0 - 128, [[512, 2], [1, 128]]), 0.0
                        )
                    if r >= 0:
                        sel = mk(P_cur[:], js * 1024 + c0, [[512, 2], [1, 512 - c0]])
                        nc.gpsimd.affine_select(
                            out=sel,
                            in_=sel,
                            compare_op=mybir.AluOpType.is_ge,
                            fill=0.0,
                            base=0,
                            pattern=[[0, 2], [1, 512 - c0]],
                            channel_multiplier=-1,
                        )
                    if ib == 0:
                        if jt == 0:
                            av_t = psAV.tile([65, 1024], F32, tag="av", name="av")
                            av = {
                                (s2, hf): av_t[:, (2 * s2 + hf) * 256 : (2 * s2 + hf + 1) * 256]
                                for s2 in range(2)
                                for hf in range(2)
                            }
                        # one accumulation group per PSUM bank (per s2):
                        # start zeroes the whole 2KB zero region, so only the
                        # very first instruction may set it
                        for s2 in range(2):
                            h = 2 * gg + s2
                            for hf in range(2):
                                if hf == 0 and jt >= 2:
                                    continue
                                nc.tensor.matmul(
                                    av[(s2, hf)],
                                    mk(vb[jt][:], h * 65, [[1, 65]]),
                                    P_cur[:, s2 * 512 + hf * 256 : s2 * 512 + (hf + 1) * 256],
                                    start=(jt == 0 and hf == 0),
                                    stop=(jt == 3 and hf == 1),
                                    skip_group_check=True,
                                )
                    elif jt % 2 == 1:
                        p = jt // 2
                        if p == 0:
                            av_t = psAV.tile([65, 1024], F32, tag="av", name="av")
                            av = {
                                (s2, hf): av_t[:, (2 * s2 + hf) * 256 : (2 * s2 + hf + 1) * 256]
                                for s2 in range(2)
                                for hf in range(2)
                            }
                        for _ in range(2):
                            if proj_pending:
                                emit_proj_chunk()
                        for s2 in range(2):
                            h = 2 * gg + s2
                            for hf in range(2):
                                if hf == 0 and p == 2 * ib + 1:
                                    continue
                                nc.tensor.matmul(
                                    av[(s2, hf)],
                                    mk(vp[p][:], h * 160, [[80, 2], [1, 65]]),
                                    mk(P_cur[:], s2 * 512 + hf * 256, [[1024, 2], [1, 256]]),
                                    start=(p == 0 and hf == 0),
                                    stop=(p == 2 * ib + 1 and hf == 1),
                                    perf_mode=DR,
                                    skip_group_check=True,
                                )
                # row end: normalize into yt
                for s2 in range(2):
                    h = 2 * gg + s2
                    for hf in range(2):
                        ytmp = ytp.tile([65, 256], BF16, tag="ytm", name="ytmp")
                        nc.vector.tensor_copy(ytmp[:], av[(s2, hf)])
                        bc = psA.tile([64, 256], F32, tag="mm", name="bc")
                        nc.tensor.matmul(
                            bc[:], ones_bc[64:65, :], ytmp[64:65, :], start=True, stop=True
                        )
                        rec = recp.tile([64, 256], F32, tag="rec", name="rec")
                        nc.vector.reciprocal_approx_fast(rec[:], bc[:])
                        dst = mk(
                            yt[64 * (h % 2) : 64 * (h % 2) + 64, :],
                            (h // 2) * t + ib * 512 + hf * 256,
                            [[1, 256]],
                        )
                        nc.vector.tensor_mul(dst, ytmp[0:64, :], rec[:])
            for ti in range(4 * ib, 4 * ib + 4):
                for cb in range(4):
                    proj_pending.append((ti, cb))
        while proj_pending:
            emit_proj_chunk()

    nc.compile()
    _NC_CACHE[key] = nc
    return nc


def _host_tables(t):
    """cos/sin [128, t] bf16, interleaved-d rows x2 heads; sin sign-folded."""
    inv_freq = (
        1.0 / (10000.0 ** (np.arange(0, HD, 2, dtype=np.float64) / np.float64(HD)))
    ).astype(np.float64)
    tt = np.arange(t, dtype=np.float64)
    freqs = tt[:, None] * inv_freq[None, :]  # [t, 32]
    cos_t = np.cos(freqs).astype(np.float32)
    sin_t = np.sin(freqs).astype(np.float32)
    cos64 = np.empty((64, t), dtype=np.float32)
    sinS64 = np.empty((64, t), dtype=np.float32)
    cos64[0::2] = cos_t.T
    cos64[1::2] = cos_t.T
    sinS64[0::2] = -sin_t.T
    sinS64[1::2] = sin_t.T
    cosT = np.concatenate([cos64, cos64], axis=0)
    sinS = np.concatenate([sinS64, sinS64], axis=0)
    return (
        np.ascontiguousarray(cosT).astype(ml_dtypes.bfloat16),
        np.ascontiguousarray(sinS).astype(ml_dtypes.bfloat16),
    )


def _ilv_perm():
    """new[2i]=old[i], new[2i+1]=old[32+i] within a head's 64 dims."""
    p = np.empty(HD, dtype=np.int64)
    p[0::2] = np.arange(32)
    p[1::2] = np.arange(32, 64)
    return p


def _dr_pack(W, out_blk):
    """[n_c, n_cols] -> [128, (cb, p, j, out_blk)] DoubleRow interleave."""
    n_c, n_cols = W.shape
    npair = n_c // 256
    nblk = n_cols // out_blk
    Wr = W.reshape(npair, 2, 128, nblk, out_blk)  # [p, j, k, cb, cq]
    return np.ascontiguousarray(
        Wr.transpose(2, 3, 0, 1, 4).reshape(128, nblk * npair * 2 * out_blk)
    )


def kernel(x, w_attn, b_attn, w_proj, b_proj):
    x = np.asarray(x, dtype=np.float32)
    w_attn = np.asarray(w_attn, dtype=np.float32)
    b_attn = np.asarray(b_attn, dtype=np.float32)
    w_proj = np.asarray(w_proj, dtype=np.float32)
    b_proj = np.asarray(b_proj, dtype=np.float32)

    t = x.shape[1]
    nc = build_nc(t)

    cosT, sinS = _host_tables(t)
    ilv = _ilv_perm()
    FP8NP = ml_dtypes.float8_e4m3

    in_maps = []
    for c in range(8):
        b = c // 2
        g = c % 2

        xT = x[b].T.astype(FP8NP)  # [1024, t]
        xt8 = np.ascontiguousarray(xT.reshape(8, 128, t).transpose(1, 0, 2).reshape(128, 8 * t))
        xTb = x[b].T[:, :512].astype(ml_dtypes.bfloat16)  # [1024, 512]
        xb16 = np.ascontiguousarray(
            xTb.reshape(8, 128, 512).transpose(1, 0, 2).reshape(128, 4096)
        )

        wq8 = np.empty((128, 4096), dtype=FP8NP)
        wk8 = np.empty((128, 4096), dtype=FP8NP)
        wqb = np.empty((128, 4096), dtype=ml_dtypes.bfloat16)
        wkb = np.empty((128, 4096), dtype=ml_dtypes.bfloat16)
        bq = np.empty((128, 4), dtype=np.float32)
        bk = np.empty((128, 4), dtype=np.float32)
        for gg in range(4):
            heads = 8 * g + 2 * gg + np.arange(2)
            cl = np.concatenate([h * HD + ilv for h in heads])  # 128 cols
            wq8[:, gg * 1024 : (gg + 1) * 1024] = _dr_pack(
                (w_attn[:, cl] * 16.0).astype(FP8NP).astype(np.float32), 128
            ).astype(FP8NP)
            wk8[:, gg * 1024 : (gg + 1) * 1024] = _dr_pack(
                (w_attn[:, C + cl] * 16.0).astype(FP8NP).astype(np.float32), 128
            ).astype(FP8NP)
            # plain bf16 packing: [128, (ci, 128)]
            wqb[:, gg * 1024 : (gg + 1) * 1024] = np.ascontiguousarray(
                (w_attn[:, cl] * 16.0)
                .reshape(8, 128, 128)
                .transpose(1, 0, 2)
                .reshape(128, 1024)
            ).astype(ml_dtypes.bfloat16)
            wkb[:, gg * 1024 : (gg + 1) * 1024] = np.ascontiguousarray(
                (w_attn[:, C + cl] * 16.0)
                .reshape(8, 128, 128)
                .transpose(1, 0, 2)
                .reshape(128, 1024)
            ).astype(ml_dtypes.bfloat16)
            bq[:, gg] = b_attn[cl] * 16.0
            bk[:, gg] = b_attn[C + cl] * 16.0

        vcols = 2 * C + g * 512 + np.arange(512)
        wv8 = _dr_pack((w_attn[:, vcols] * 16.0).astype(FP8NP).astype(np.float32), 256).astype(
            FP8NP
        )
        # plain bf16 packing for early keys: [128, (ci, cb, 256)]
        wvb = np.ascontiguousarray(
            (w_attn[:, vcols] * 16.0)
            .reshape(8, 128, 2, 256)
            .transpose(1, 0, 2, 3)
            .reshape(128, 4096)
        ).astype(ml_dtypes.bfloat16)
        bv = b_attn[vcols] * 16.0

        # proj: bf16, plain (ci, cb) blocks: [128, (ci, cb, 256)]
        Wp = (w_proj[g * 512 : (g + 1) * 512, :] * 16.0).astype(ml_dtypes.bfloat16)
        wpb = np.ascontiguousarray(
            Wp.reshape(4, 128, 4, 256).transpose(1, 0, 2, 3).reshape(128, 4096)
        )

        in_maps.append(
            {
                "xt": xt8,
                "xb": xb16,
                "wq": wq8,
                "wk": wk8,
                "wv": wv8,
                "wp": wpb,
                "wqb": wqb,
                "wkb": wkb,
                "wvb": wvb,
                "bq": bq,
                "bk": bk,
                "bv": bv.astype(np.float32),
                "cosT": cosT,
                "sinS": sinS,
            }
        )

    res = run_bass_kernel_spmd(nc, in_maps, core_ids=list(range(8)))
    global LAST_RESULTS
    LAST_RESULTS = res

    out = np.empty((B, t, C), dtype=np.float32)
    for b in range(B):
        acc = (
            res.results[2 * b]["out"].astype(np.float64)
            + res.results[2 * b + 1]["out"].astype(np.float64)
        ) * (1.0 / 128.0) + b_proj.astype(np.float64)[None, :]
        out[b] = acc.astype(np.float32)
    return out


# revision 43
# speedup vs baseline: 1.0071x; 1.0071x over previous
"""Trainium2 Bass kernel for causal multi-head attention with RoPE (v2, fp8).

Problem: B=4, T=2048, C=1024, 16 heads, head_dim=64, fp32.
Sharding over 8 cores: core c handles batch c//2 and heads [8*(c%2), 8*(c%2)+8).
Each core computes a [T, C] partial of the output projection; the host sums
the two partials per batch, scales by 1/128 and adds b_proj.

Precision plan (validated by numpy emulation):
  - QKV GEMMs: fp8 e4m3 DoubleRow (256-deep contraction per instruction),
    weights host-scaled x16 into the fp8 normal range.
  - S = q^T k: plain fp8 (contraction is only 64, DoubleRow gains nothing).
  - P = exp((S - 2*2048)/2048): ACT engine straight from PSUM into fp8.
  - AV: fp8 DoubleRow over key-tile pairs; denominator via a constant-2.0
    column appended to V (row 64 of the AV output).
  - proj: bf16 (fp8 here would put ~4% RMS directly on the output).
  The net 128x output scale (16 * 8) is divided out on the host.

Head layout per core: local head h in [0,8) -> tile gg=h//2, partition base
64*(h%2); head dims stored interleaved (RoPE rotation pairs adjacent).

Self-contained: hardcodes shapes; needs numpy + ml_dtypes + concourse.
"""

import numpy as np
import ml_dtypes
from contextlib import ExitStack

import concourse.bass as bass
import concourse.tile as tile
from concourse import bacc, mybir
from concourse.bass_utils import run_bass_kernel_spmd

F32 = mybir.dt.float32
BF16 = mybir.dt.bfloat16
FP8 = mybir.dt.float8e4
AF = mybir.ActivationFunctionType
DR = mybir.MatmulPerfMode.DoubleRow

B, T, C = 4, 2048, 1024
N_HEAD = 16
HD = 64
HG = 8  # heads per core
SCALE_EFF = 1.0 / (np.sqrt(HD) * 256.0)  # wq,wk each scaled x16
EXP_BIAS = -2.0

_NC_CACHE = {}
LAST_RESULTS = None


def _pair_swap_mask():
    m = []
    for i in range(16):
        m += [2 * i + 1, 2 * i]
    return m


def mk(ap, off, dims):
    """AP with the same partition dim but custom free dims, offset in elems."""
    return bass.AP(tensor=ap.tensor, offset=ap.offset + off, ap=[list(ap.ap[0])] + dims)


def build_nc(t=T):
    key = t
    if key in _NC_CACHE:
        return _NC_CACHE[key]

    n_tt = t // 128  # 16 key tiles
    n_tb = t // 512  # 4 query blocks
    n_pair = n_tt // 2  # 8 key-tile pairs

    nc = bacc.Bacc("TRN2", target_bir_lowering=False, debug=False, num_devices=8)

    xt_d = nc.dram_tensor("xt", [128, 8 * t], FP8, kind="ExternalInput").ap()
    xb_d = nc.dram_tensor("xb", [128, 4096], BF16, kind="ExternalInput").ap()
    wq_d = nc.dram_tensor("wq", [128, 4096], FP8, kind="ExternalInput").ap()
    wk_d = nc.dram_tensor("wk", [128, 4096], FP8, kind="ExternalInput").ap()
    wv_d = nc.dram_tensor("wv", [128, 4096], FP8, kind="ExternalInput").ap()
    wp_d = nc.dram_tensor("wp", [128, 4096], BF16, kind="ExternalInput").ap()
    wqb_d = nc.dram_tensor("wqb", [128, 4096], BF16, kind="ExternalInput").ap()
    wkb_d = nc.dram_tensor("wkb", [128, 4096], BF16, kind="ExternalInput").ap()
    wvb_d = nc.dram_tensor("wvb", [128, 4096], BF16, kind="ExternalInput").ap()
    bq_d = nc.dram_tensor("bq", [128, 4], F32, kind="ExternalInput").ap()
    bk_d = nc.dram_tensor("bk", [128, 4], F32, kind="ExternalInput").ap()
    bv_d = nc.dram_tensor("bv", [512], F32, kind="ExternalInput").ap()
    cos_d = nc.dram_tensor("cosT", [128, t], BF16, kind="ExternalInput").ap()
    sin_d = nc.dram_tensor("sinS", [128, t], BF16, kind="ExternalInput").ap()
    out_d = nc.dram_tensor("out", [t, C], F32, kind="ExternalOutput").ap()

    shuf = _pair_swap_mask()

    with tile.TileContext(nc) as tc, ExitStack() as ctx:
        # ---------------- persistent SBUF ----------------
        persist = ctx.enter_context(tc.tile_pool(name="persist", bufs=1))
        xt = persist.tile([128, 8 * t], FP8, tag="xt", name="xt")
        xb = persist.tile([128, 4096], BF16, tag="xb", name="xb")  # x^T keys<512
        qt8 = [persist.tile([128, t], BF16, tag=f"qt{g}", name=f"qt{g}") for g in range(4)]
        kt8 = [persist.tile([128, t], BF16, tag=f"kt{g}", name=f"kt{g}") for g in range(4)]
        vp = [persist.tile([128, 1280], FP8, tag=f"v{i}", name=f"v{i}") for i in range(n_pair)]
        vb = [persist.tile([128, 520], BF16, tag=f"vb{i}", name=f"vb{i}") for i in range(4)]
        yt = persist.tile([128, 4 * t], BF16, tag="yt", name="yt")
        wq_sb = persist.tile([128, 4096], FP8, tag="wq", name="wq")
        wk_sb = persist.tile([128, 4096], FP8, tag="wk", name="wk")
        wv_sb = persist.tile([128, 4096], FP8, tag="wv", name="wv")
        wp_sb = persist.tile([128, 4096], BF16, tag="wp", name="wp")
        wqb_sb = persist.tile([128, 4096], BF16, tag="wqb", name="wqb")
        wkb_sb = persist.tile([128, 4096], BF16, tag="wkb", name="wkb")
        wvb_sb = persist.tile([128, 4096], BF16, tag="wvb", name="wvb")
        cos_sb = persist.tile([128, t], BF16, tag="cos", name="cos")
        sin_sb = persist.tile([128, t], BF16, tag="sin", name="sin")
        bq_sb = persist.tile([128, 4], F32, tag="bq", name="bq")
        bk_sb = persist.tile([128, 4], F32, tag="bk", name="bk")
        bv_sb = persist.tile([128, 512], F32, tag="bv", name="bv")
        ones_bc = persist.tile([128, 64], BF16, tag="ones", name="ones")
        nbias = persist.tile([128, 1], F32, tag="nbias", name="nbias")

        scr = ctx.enter_context(tc.tile_pool(name="scr", bufs=3))
        rtp = ctx.enter_context(tc.tile_pool(name="rtp", bufs=2))
        pP = ctx.enter_context(tc.tile_pool(name="pP", bufs=3))
        pPb = ctx.enter_context(tc.tile_pool(name="pPb", bufs=2))
        ytp = ctx.enter_context(tc.tile_pool(name="ytp", bufs=8))
        recp = ctx.enter_context(tc.tile_pool(name="recp", bufs=8))
        osb = ctx.enter_context(tc.tile_pool(name="osb", bufs=4))

        psA = ctx.enter_context(tc.tile_pool(name="psA", bufs=2, space="PSUM"))
        psS = ctx.enter_context(tc.tile_pool(name="psS", bufs=2, space="PSUM"))
        psAV = ctx.enter_context(tc.tile_pool(name="psAV", bufs=1, space="PSUM"))

        nc.vector.memset(ones_bc[:], 1.0)
        nc.vector.memset(nbias[:], EXP_BIAS)

        # ---------------- input DMAs, spread across queues, by first use ----
        # per-queue DMA bandwidth is ~125 GB/s; the three queues run in
        # parallel, so spread the 11 MB of inputs and order by first need.
        nc.gpsimd.dma_start(wvb_sb[:, 0:2048], wvb_d[:, 0:2048])
        nc.sync.dma_start(xb[:, 0:2048], xb_d[:, 0:2048])
        nc.scalar.dma_start(xb[:, 2048:4096], xb_d[:, 2048:4096])
        nc.gpsimd.dma_start(wvb_sb[:, 2048:4096], wvb_d[:, 2048:4096])
        nc.gpsimd.dma_start(wv_sb[:], wv_d)
        nc.sync.dma_start(xt[:, 0:4096], xt_d[:, 0:4096])
        nc.scalar.dma_start(xt[:, 4096 : 2 * 4096], xt_d[:, 4096 : 2 * 4096])
        nc.gpsimd.dma_start(xt[:, 2 * 4096 : 3 * 4096], xt_d[:, 2 * 4096 : 3 * 4096])
        nc.sync.dma_start(xt[:, 3 * 4096 : 4 * 4096], xt_d[:, 3 * 4096 : 4 * 4096])
        nc.scalar.dma_start(bq_sb[:], bq_d)
        nc.scalar.dma_start(bk_sb[:], bk_d)
        nc.scalar.dma_start(wqb_sb[:], wqb_d)
        nc.sync.dma_start(wq_sb[:], wq_d)
        nc.sync.dma_start(cos_sb[:], cos_d)
        nc.sync.dma_start(sin_sb[:], sin_d)
        nc.gpsimd.dma_start(wkb_sb[:], wkb_d)
        nc.scalar.dma_start(wk_sb[:], wk_d)
        nc.sync.dma_start(
            bv_sb[:], bass.AP(tensor=bv_d.tensor, offset=0, ap=[[0, 128], [1, 512]])
        )
        nc.gpsimd.dma_start(wp_sb[:], wp_d)

        # 2.0 columns for the softmax denominator (d=64 slot per (h, j))
        for p in range(n_pair):
            nc.gpsimd.memset(mk(vp[p][:], 64, [[160, 8], [80, 2]]), 2.0)
        for i in range(4):
            nc.gpsimd.memset(mk(vb[i][:], 64, [[65, 8]]), 2.0)

        # ---------------- V: per key tile, 2 channel blocks ----------------
        # keys < 512: accurate bf16 GEMM, written to both vb (bf16) and vp
        # (fp8); keys >= 512: fp8 DoubleRow.
        def emit_v(kt):
            for cb in range(2):
                ps = psA.tile([128, 256], F32, tag="mm", name="psv")
                if kt < 4:
                    for ci in range(8):
                        nc.tensor.matmul(
                            ps[:],
                            mk(xb[:], ci * 512 + kt * 128, [[1, 128]]),
                            mk(wvb_sb[:], cb * 2048 + ci * 256, [[1, 256]]),
                            start=(ci == 0),
                            stop=(ci == 7),
                        )
                    nc.vector.tensor_add(
                        mk(vb[kt][:], (4 * cb) * 65, [[65, 4], [1, 64]]),
                        mk(ps[:], 0, [[64, 4], [1, 64]]),
                        mk(bv_sb[:], cb * 256, [[64, 4], [1, 64]]),
                    )
                else:
                    for p in range(4):
                        nc.tensor.matmul(
                            ps[:],
                            mk(xt[:], (2 * p) * t + kt * 128, [[t, 2], [1, 128]]),
                            mk(wv_sb[:], cb * 2048 + p * 512, [[256, 2], [1, 256]]),
                            start=(p == 0),
                            stop=(p == 3),
                            perf_mode=DR,
                        )
                nc.vector.tensor_add(
                    mk(vp[kt // 2][:], (4 * cb) * 160 + (kt % 2) * 80, [[160, 4], [1, 64]]),
                    mk(ps[:], 0, [[64, 4], [1, 64]]),
                    mk(bv_sb[:], cb * 256, [[64, 4], [1, 64]]),
                )

        # ---------------- Q, K with RoPE ----------------
        def qk_chunks(w_sb, wb_sb, b_sb, dst, gg):
            """Emitter closures: 8 per-nb GEMM chunks + 1 RoPE chunk."""
            box = {}

            def mk_nb(nb):
                def f():
                    if "q" not in box:
                        box["q"] = scr.tile([128, t], BF16, tag="scr", name="qscr")
                    qscr = box["q"]
                    ps = psA.tile([128, 256], F32, tag="mm", name="psq")
                    if nb < 2:
                        # queries < 512: accurate bf16 GEMM
                        for ci in range(8):
                            nc.tensor.matmul(
                                ps[:],
                                mk(wb_sb[:], gg * 1024 + ci * 128, [[1, 128]]),
                                mk(xb[:], ci * 512 + nb * 256, [[1, 256]]),
                                start=(ci == 0),
                                stop=(ci == 7),
                            )
                    else:
                        for p in range(4):
                            nc.tensor.matmul(
                                ps[:],
                                mk(w_sb[:], gg * 1024 + p * 256, [[128, 2], [1, 128]]),
                                mk(xt[:], (2 * p) * t + nb * 256, [[t, 2], [1, 256]]),
                                start=(p == 0),
                                stop=(p == 3),
                                perf_mode=DR,
                            )
                    nc.scalar.add(
                        qscr[:, nb * 256 : (nb + 1) * 256], ps[:], b_sb[:, gg : gg + 1]
                    )

                return f

            def rope():
                qscr = box["q"]
                tmp = rtp.tile([128, t], BF16, tag="rt", name="rtmp")
                nc.vector.stream_shuffle(tmp[:], qscr[:], shuf)
                nc.vector.tensor_mul(tmp[:], tmp[:], sin_sb[:])
                nc.vector.tensor_mul(qscr[:], qscr[:], cos_sb[:])
                nc.vector.tensor_add(dst[:], qscr[:], tmp[:])

            return [mk_nb(nb) for nb in range(8)] + [rope]

        def all_qk_chunks(gg):
            return qk_chunks(wq_sb, wqb_sb, bq_sb, qt8[gg], gg) + qk_chunks(
                wk_sb, wkb_sb, bk_sb, kt8[gg], gg
            )

        # Emission order: V-early (xb-dependent), V-late part 1, all Q/K,
        # V-late part 2 (PE work covering the gg=3 RoPE drain on DVE, since
        # cross-engine waits are in-order), then attention.
        for kt in range(10):
            emit_v(kt)
        for gg in range(4):
            for ch in all_qk_chunks(gg):
                ch()
        for kt in range(10, n_tt):
            emit_v(kt)

        # ---------------- attention, ib-major with proj interleaved ----------
        proj_pending = []
        dma_i = [0]

        def emit_proj_chunk():
            ti, cb = proj_pending.pop(0)
            ps = psA.tile([128, 256], F32, tag="mm", name="psp")
            for ci in range(4):
                nc.tensor.matmul(
                    ps[:],
                    mk(yt[:], ci * t + ti * 128, [[1, 128]]),
                    mk(wp_sb[:], ci * 1024 + cb * 256, [[1, 256]]),
                    start=(ci == 0),
                    stop=(ci == 3),
                )
            o = osb.tile([128, 256], F32, tag="o", name="o")
            nc.vector.tensor_copy(o[:], ps[:])
            eng = nc.sync if dma_i[0] % 2 == 0 else nc.scalar
            dma_i[0] += 1
            eng.dma_start(out_d[ti * 128 : (ti + 1) * 128, cb * 256 : (cb + 1) * 256], o[:])

        def pop_fill(n):
            for _ in range(n):
                if proj_pending:
                    emit_proj_chunk()
                else:
                    break

        for ib in range(n_tb):
            for gg in range(4):
                P_cur = None
                av = None
                n_jt = 4 * ib + 4
                for jt in range(n_jt):
                    r = jt - 4 * ib
                    c0 = max(0, 128 * r)
                    sq = psS.tile([128, 1024], F32, tag="s", name="sq")
                    for s2 in range(2):
                        nc.tensor.matmul(
                            sq[:, s2 * 512 + c0 : (s2 + 1) * 512],
                            kt8[gg][64 * s2 : 64 * s2 + 64, jt * 128 : (jt + 1) * 128],
                            qt8[gg][64 * s2 : 64 * s2 + 64, ib * 512 + c0 : (ib + 1) * 512],
                            start=True,
                            stop=True,
                            tile_position=(64 * s2, 0),
                        )
                    if ib == 0:
                        P_cur = pPb.tile([128, 1024], BF16, tag="pb", name="Pb")
                        js = 0
                    else:
                        if jt % 2 == 0:
                            P_cur = pP.tile([128, 2048], FP8, tag="p", name="P")
                        js = jt % 2
                    nc.scalar.activation(
                        mk(P_cur[:], js * 1024 + c0, [[512, 2], [1, 512 - c0]]),
                        mk(sq[:], c0, [[512, 2], [1, 512 - c0]]),
                        AF.Exp,
                        scale=SCALE_EFF,
                        bias=nbias[:],
                    )
                    if r in (1, 3):
                        # zero the consumed-but-unwritten 128 cols below c0
                        nc.gpsimd.memset(
                            mk(P_cur[:], js * 1024 + c0 - 128, [[512, 2], [1, 128]]), 0.0
                        )
                    if r >= 0:
                        sel = mk(P_cur[:], js * 1024 + c0, [[512, 2], [1, 512 - c0]])
                        nc.gpsimd.affine_select(
                            out=sel,
                            in_=sel,
                            compare_op=mybir.AluOpType.is_ge,
                            fill=0.0,
                            base=0,
                            pattern=[[0, 2], [1, 512 - c0]],
                            channel_multiplier=-1,
                        )
                    if ib == 0:
                        if jt == 0:
                            av_t = psAV.tile([65, 1024], F32, tag="av", name="av")
                            av = {
                                (s2, hf): av_t[:, (2 * s2 + hf) * 256 : (2 * s2 + hf + 1) * 256]
                                for s2 in range(2)
                                for hf in range(2)
                            }
                        pop_fill(2)
                        # one accumulation group per PSUM bank (per s2):
                        # start zeroes the whole 2KB zero region, so only the
                        # very first instruction may set it
                        for s2 in range(2):
                            h = 2 * gg + s2
                            for hf in range(2):
                                if hf == 0 and jt >= 2:
                                    continue
                                nc.tensor.matmul(
                                    av[(s2, hf)],
                                    mk(vb[jt][:], h * 65, [[1, 65]]),
                                    P_cur[:, s2 * 512 + hf * 256 : s2 * 512 + (hf + 1) * 256],
                                    start=(jt == 0 and hf == 0),
                                    stop=(jt == 3 and hf == 1),
                                    skip_group_check=True,
                                )
                    elif jt % 2 == 1:
                        p = jt // 2
                        if p == 0:
                            av_t = psAV.tile([65, 1024], F32, tag="av", name="av")
                            av = {
                                (s2, hf): av_t[:, (2 * s2 + hf) * 256 : (2 * s2 + hf + 1) * 256]
                                for s2 in range(2)
                                for hf in range(2)
                            }
                        pop_fill(2)
                        for s2 in range(2):
                            h = 2 * gg + s2
                            for hf in range(2):
                                if hf == 0 and p == 2 * ib + 1:
                                    continue
                                nc.tensor.matmul(
                                    av[(s2, hf)],
                                    mk(vp[p][:], h * 160, [[80, 2], [1, 65]]),
                                    mk(P_cur[:], s2 * 512 + hf * 256, [[1024, 2], [1, 256]]),
                                    start=(p == 0 and hf == 0),
                                    stop=(p == 2 * ib + 1 and hf == 1),
                                    perf_mode=DR,
                                    skip_group_check=True,
                                )
                # row end: normalize into yt
                for s2 in range(2):
                    h = 2 * gg + s2
                    for hf in range(2):
                        ytmp = ytp.tile([65, 256], BF16, tag="ytm", name="ytmp")
                        nc.vector.tensor_copy(ytmp[:], av[(s2, hf)])
                        bc = psA.tile([64, 256], F32, tag="mm", name="bc")
                        nc.tensor.matmul(
                            bc[:], ones_bc[64:65, :], ytmp[64:65, :], start=True, stop=True
                        )
                        rec = recp.tile([64, 256], F32, tag="rec", name="rec")
                        nc.vector.reciprocal_approx_fast(rec[:], bc[:])
                        dst = mk(
                            yt[64 * (h % 2) : 64 * (h % 2) + 64, :],
                            (h // 2) * t + ib * 512 + hf * 256,
                            [[1, 256]],
                        )
                        nc.vector.tensor_mul(dst, ytmp[0:64, :], rec[:])
            for ti in range(4 * ib, 4 * ib + 4):
                for cb in range(4):
                    proj_pending.append((ti, cb))
        while proj_pending:
            emit_proj_chunk()

    nc.compile()
    _NC_CACHE[key] = nc
    return nc


def _host_tables(t):
    """cos/sin [128, t] bf16, interleaved-d rows x2 heads; sin sign-folded."""
    inv_freq = (
        1.0 / (10000.0 ** (np.arange(0, HD, 2, dtype=np.float64) / np.float64(HD)))
    ).astype(np.float64)
    tt = np.arange(t, dtype=np.float64)
    freqs = tt[:, None] * inv_freq[None, :]  # [t, 32]
    cos_t = np.cos(freqs).astype(np.float32)
    sin_t = np.sin(freqs).astype(np.float32)
    cos64 = np.empty((64, t), dtype=np.float32)
    sinS64 = np.empty((64, t), dtype=np.float32)
    cos64[0::2] = cos_t.T
    cos64[1::2] = cos_t.T
    sinS64[0::2] = -sin_t.T
    sinS64[1::2] = sin_t.T
    cosT = np.concatenate([cos64, cos64], axis=0)
    sinS = np.concatenate([sinS64, sinS64], axis=0)
    return (
        np.ascontiguousarray(cosT).astype(ml_dtypes.bfloat16),
        np.ascontiguousarray(sinS).astype(ml_dtypes.bfloat16),
    )


def _ilv_perm():
    """new[2i]=old[i], new[2i+1]=old[32+i] within a head's 64 dims."""
    p = np.empty(HD, dtype=np.int64)
    p[0::2] = np.arange(32)
    p[1::2] = np.arange(32, 64)
    return p


def _dr_pack(W, out_blk):
    """[n_c, n_cols] -> [128, (cb, p, j, out_blk)] DoubleRow interleave."""
    n_c, n_cols = W.shape
    npair = n_c // 256
    nblk = n_cols // out_blk
    Wr = W.reshape(npair, 2, 128, nblk, out_blk)  # [p, j, k, cb, cq]
    return np.ascontiguousarray(
        Wr.transpose(2, 3, 0, 1, 4).reshape(128, nblk * npair * 2 * out_blk)
    )


def kernel(x, w_attn, b_attn, w_proj, b_proj):
    x = np.asarray(x, dtype=np.float32)
    w_attn = np.asarray(w_attn, dtype=np.float32)
    b_attn = np.asarray(b_attn, dtype=np.float32)
    w_proj = np.asarray(w_proj, dtype=np.float32)
    b_proj = np.asarray(b_proj, dtype=np.float32)

    t = x.shape[1]
    nc = build_nc(t)

    cosT, sinS = _host_tables(t)
    ilv = _ilv_perm()
    FP8NP = ml_dtypes.float8_e4m3

    in_maps = []
    for c in range(8):
        b = c // 2
        g = c % 2

        xT = x[b].T.astype(FP8NP)  # [1024, t]
        xt8 = np.ascontiguousarray(xT.reshape(8, 128, t).transpose(1, 0, 2).reshape(128, 8 * t))
        # xb layout [128, (ci 8, 512)]: x^T cols t<512
        xTb = x[b].T[:, :512].astype(ml_dtypes.bfloat16)  # [1024 c, 512]
        xb16 = np.ascontiguousarray(
            xTb.reshape(8, 128, 512).transpose(1, 0, 2).reshape(128, 4096)
        )

        wq8 = np.empty((128, 4096), dtype=FP8NP)
        wk8 = np.empty((128, 4096), dtype=FP8NP)
        wqb = np.empty((128, 4096), dtype=ml_dtypes.bfloat16)
        wkb = np.empty((128, 4096), dtype=ml_dtypes.bfloat16)
        bq = np.empty((128, 4), dtype=np.float32)
        bk = np.empty((128, 4), dtype=np.float32)
        for gg in range(4):
            heads = 8 * g + 2 * gg + np.arange(2)
            cl = np.concatenate([h * HD + ilv for h in heads])  # 128 cols
            wq8[:, gg * 1024 : (gg + 1) * 1024] = _dr_pack(
                (w_attn[:, cl] * 16.0).astype(FP8NP).astype(np.float32), 128
            ).astype(FP8NP)
            wk8[:, gg * 1024 : (gg + 1) * 1024] = _dr_pack(
                (w_attn[:, C + cl] * 16.0).astype(FP8NP).astype(np.float32), 128
            ).astype(FP8NP)
            # plain bf16 packing: [128, (ci, 128)]
            wqb[:, gg * 1024 : (gg + 1) * 1024] = np.ascontiguousarray(
                (w_attn[:, cl] * 16.0)
                .reshape(8, 128, 128)
                .transpose(1, 0, 2)
                .reshape(128, 1024)
            ).astype(ml_dtypes.bfloat16)
            wkb[:, gg * 1024 : (gg + 1) * 1024] = np.ascontiguousarray(
                (w_attn[:, C + cl] * 16.0)
                .reshape(8, 128, 128)
                .transpose(1, 0, 2)
                .reshape(128, 1024)
            ).astype(ml_dtypes.bfloat16)
            bq[:, gg] = b_attn[cl] * 16.0
            bk[:, gg] = b_attn[C + cl] * 16.0

        vcols = 2 * C + g * 512 + np.arange(512)
        wv8 = _dr_pack((w_attn[:, vcols] * 16.0).astype(FP8NP).astype(np.float32), 256).astype(
            FP8NP
        )
        # plain bf16 packing for early keys: [128, (cb, ci, 256)]
        wvb = np.ascontiguousarray(
            (w_attn[:, vcols] * 16.0)
            .reshape(8, 128, 2, 256)
            .transpose(1, 2, 0, 3)
            .reshape(128, 4096)
        ).astype(ml_dtypes.bfloat16)
        bv = b_attn[vcols] * 16.0

        # proj: bf16, plain (ci, cb) blocks: [128, (ci, cb, 256)]
        Wp = (w_proj[g * 512 : (g + 1) * 512, :] * 16.0).astype(ml_dtypes.bfloat16)
        wpb = np.ascontiguousarray(
            Wp.reshape(4, 128, 4, 256).transpose(1, 0, 2, 3).reshape(128, 4096)
        )

        in_maps.append(
            {
                "xt": xt8,
                "xb": xb16,
                "wq": wq8,
                "wk": wk8,
                "wv": wv8,
                "wp": wpb,
                "wqb": wqb,
                "wkb": wkb,
                "wvb": wvb,
                "bq": bq,
                "bk": bk,
                "bv": bv.astype(np.float32),
                "cosT": cosT,
                "sinS": sinS,
            }
        )

    res = run_bass_kernel_spmd(nc, in_maps, core_ids=list(range(8)))
    global LAST_RESULTS
    LAST_RESULTS = res

    out = np.empty((B, t, C), dtype=np.float32)
    for b in range(B):
        acc = (
            res.results[2 * b]["out"].astype(np.float64)
            + res.results[2 * b + 1]["out"].astype(np.float64)
        ) * (1.0 / 128.0) + b_proj.astype(np.float64)[None, :]
        out[b] = acc.astype(np.float32)
    return out
